# revision 40
# baseline (speedup 1.0000x reference)
"""Trainium2 Bass kernel for nn_BiGlobal_MPCMFuse (8 NeuronCores, SPMD).

Math (see reference):
    pcm_s  = min over 4 direction pairs of (cen[x+d]-cen[x])*(cen[x-d]-cen[x]),
             d in {(s,0),(s,s),(s,-s),(0,s)}, circular shifts, s in {13, 17}
    pcmN   = BN(pcm_s)  (train-mode BN over (B,H,W) per channel)
    wei    = SE-attention on the (H,W)-pooled pcmN  -> per-(b,c) sigmoid weights
    out    = td_wei * pcm13N + bu_wei * pcm17N

Device strategy (2 NEFF launches + tiny host glue):
  - Shard H across the 8 cores (48 rows each + 17-row halo), all 256 (b,c)
    planes per core.  BN/SE stats are plain sums -> partials combined on host.
  - Layout: partitions = planes (2 blocks of 128), free dim = [rows, cols]
    with halo'd cols so every circular shift is a plain 2D AP offset.
  - bf16 compute on the Vector engine (tensor_tensor @2x mode).  A second
    one-element-shifted copy of the input (cenO) keeps odd-element shifts
    4-byte aligned so the 2x mode is retained for all 15 ops per scale.
  - Pass A is a uniform slab pipeline: 12-output-row slabs (46 input rows),
    cE double-buffered, cO single-buffered.  Per-slab op order puts the
    axis-aligned pairs (which read only cE) first so the next slab's cO
    reload hides under the odd-pair tail of the current slab.
  - Pass A emits raw pcm13/pcm17 (bf16) + per-plane sum/sumsq partials
    (ScalarE activation accum).  Host computes BN affine + SE MLP exactly
    (float64) and folds everything into per-plane coefficients:
        out = A13[p]*pcm13 + A17[p]*pcm17 + D[p]
  - Pass B applies that combine fully on the DVE (tensor_scalar @4x +
    scalar_tensor_tensor @2x), emitting bf16; host upcasts to f32.
"""

import os
import sys

import numpy as np

for _p in ("/opt/trn_rl_repo",):
    if _p not in sys.path and os.path.isdir(_p):
        sys.path.insert(0, _p)

import ml_dtypes  # noqa: E402

BF16 = ml_dtypes.bfloat16

B, C, H, W = 4, 64, 384, 384
IC = C // 2
NCORES = 8
P = B * C            # 256 planes
ROWS = H // NCORES   # 48 rows per core
RHALO = 17           # row halo each side (max |shift| 17)
CHALO = 18           # col halo each side
SR = ROWS + 2 * RHALO   # 84 stored rows
SW = W + 2 * CHALO      # 420 stored cols
NBLK = 2             # 256 planes / 128 partitions
CR = 12              # chunk rows (pass A); 48/12 = 4 steps
NSTEPS = ROWS // CR
SLAB = CR + 2 * RHALO   # 46 input rows per slab
CR_B = 24            # chunk rows (pass B); 2 steps
EPS = 1e-5
SCALES = (13, 17)

_cache = {}


def _build_pass_a():
    import concourse.bacc as bacc
    import concourse.tile as tile
    from concourse import mybir

    nc = bacc.Bacc()
    bf = mybir.dt.bfloat16
    f32 = mybir.dt.float32

    cenE = nc.declare_dram_parameter("cenE", [NBLK, 128, SR, SW], bf, isOutput=False)
    m13 = nc.declare_dram_parameter("m13", [NBLK, 128, ROWS, W], bf, isOutput=True)
    m17 = nc.declare_dram_parameter("m17", [NBLK, 128, ROWS, W], bf, isOutput=True)
    # per (block, partition): [sum13, sq13, sum17, sq17] x NSTEPS
    stats = nc.declare_dram_parameter(
        "stats", [NBLK, 128, 4, NSTEPS], f32, isOutput=True
    )
    m_out = {13: m13, 17: m17}

    import concourse.bass as bass

    s0, s1 = SCALES          # 13, 17
    ds = s1 - s0             # 4: window stride between the two scales

    with tile.TileContext(nc) as tc:
        with (
            tc.tile_pool(name="cep", bufs=2) as cep,
            tc.tile_pool(name="cop", bufs=1) as cop,
            tc.tile_pool(name="mbuf", bufs=2) as mbuf,
            tc.tile_pool(name="work", bufs=1) as work,
            tc.tile_pool(name="accp", bufs=1) as accp,
            tc.tile_pool(name="sqp", bufs=1) as sqp,
        ):
            # Every shifted read falls in one of three disjoint row bands of
            # the 46-row slab: A=[0,16) (dy<0), B=[17,29) (dy=0, center),
            # C=[30,46) (dy>0).  Rows 16 and 29 are never read.  Separate
            # tiles per band give fine-grained DMA->compute dependencies
            # (Tile tracks deps per tile): the first op waits only on its
            # own 1.7MB band, not the whole 9.7MB slab.
            BANDS = {"A": (0, 16), "B": (17, 12), "C": (30, 16)}

            for blk in range(NBLK):
                acc = accp.tile([128, 4, NSTEPS], f32, tag="acc", name=f"acc{blk}")
                srcf = cenE[blk].rearrange("p a b -> p (a b)")
                for step in range(NSTEPS):
                    r0 = step * CR      # SR-index of first slab row
                    cE, cO = {}, {}
                    for bname, (bs, bn) in BANDS.items():
                        cE[bname] = cep.tile(
                            [128, bn, SW], bf, tag=f"cE{bname}",
                            name=f"cE{bname}{blk}_{step}",
                        )
                        cO[bname] = cop.tile(
                            [128, bn, SW], bf, tag=f"cO{bname}",
                            name=f"cO{bname}{blk}_{step}",
                        )
                    # Load order: A bands first (axis/diag dM windows — the
                    # slab's first ops), then B (center), then C, then the
                    # cO bands (read later in the slab).
                    for bname, (bs, bn) in BANDS.items():
                        nc.sync.dma_start(
                            out=cE[bname],
                            in_=cenE[blk][:, r0 + bs : r0 + bs + bn, :],
                        )
                    # cO emission order matches first use: the diagonal
                    # pairs read bands C and A first, the column pair
                    # (band B) runs last in the slab.
                    for bname in ("C", "A", "B"):
                        bs, bn = BANDS[bname]
                        # cO[p, i] = cen_flat[p, (r0+bs)*SW + i + 1]: the
                        # parity copy of the band.  On the very last band the
                        # +1 shift would run past srcf's end — clamp and
                        # leave the final element garbage (never read).
                        b0 = (r0 + bs) * SW
                        end = min(b0 + bn * SW + 1, SR * SW)
                        nc.sync.dma_start(
                            out=cO[bname].rearrange("p a b -> p (a b)")[
                                :, 0 : end - b0 - 1
                            ],
                            in_=srcf[:, b0 + 1 : end],
                        )

                    def rd2(dy, dx):
                        # Scale-merged read: [128, 2, CR, W] AP covering the
                        # (dy,dx)-shifted window for BOTH scales, where
                        # (dy,dx) is the s=13 shift and the s=17 window sits
                        # at a constant element offset delta away.
                        sy, sx = (0 if dy == 0 else (1 if dy > 0 else -1),
                                  0 if dx == 0 else (1 if dx > 0 else -1))
                        bname = "B" if dy == 0 else ("C" if dy > 0 else "A")
                        bs, _ = BANDS[bname]
                        r = RHALO + dy - bs
                        if dx % 2 == 0:
                            t, c = cE[bname], CHALO + dx
                        else:
                            t, c = cO[bname], CHALO + dx - 1
                        delta = ds * (sy * SW + sx)
                        a = t[:, r : r + CR, c : c + W]
                        return bass.AP(
                            tensor=a.tensor,
                            offset=a.offset,
                            ap=[a.ap[0], [delta, 2], a.ap[1], a.ap[2]],
                        )

                    cen2 = rd2(0, 0)     # center, broadcast over the scale dim
                    M = mbuf.tile([128, 2, CR, W], bf, tag="M", name=f"M_{blk}_{step}")
                    dP = work.tile([128, 2, CR, W], bf, tag="dP", name="dP", bufs=1)
                    dM = work.tile([128, 2, CR, W], bf, tag="dM", name="dM", bufs=1)
                    # Axis pair first: it reads only cE, so the next slab's
                    # cO DMA can still be in flight.  dM before dP: its
                    # windows sit in the first cE piece.
                    nc.vector.tensor_sub(dM, rd2(-s0, 0), cen2)
                    nc.vector.tensor_sub(dP, rd2(s0, 0), cen2)
                    nc.vector.tensor_mul(M, dP, dM)
                    # Odd pairs (read cO) last.  Product is computed in
                    # place into dP (legal on DVE, same-position aliasing).
                    last_slab = blk == NBLK - 1 and step == NSTEPS - 1
                    for dy, dx in ((s0, s0), (s0, -s0), (0, s0)):
                        if last_slab and (dy, dx) == (0, s0):
                            # Final pair of the whole kernel: de-merge the
                            # scales so M[:,0] finalizes early and its stats
                            # (ScalarE) overlap M[:,1]'s last three DVE ops,
                            # shortening the kernel tail.
                            for si, sc in enumerate(SCALES):
                                a13 = rd2(0, s0)
                                aP = bass.AP(tensor=a13.tensor,
                                             offset=a13.offset + si * ds,
                                             ap=[a13.ap[0], a13.ap[2], a13.ap[3]])
                                a13m = rd2(0, -s0)
                                aM = bass.AP(tensor=a13m.tensor,
                                             offset=a13m.offset - si * ds,
                                             ap=[a13m.ap[0], a13m.ap[2], a13m.ap[3]])
                                cen1 = cE["B"][:, 0:CR, CHALO : CHALO + W]
                                nc.vector.tensor_sub(dP[:, si], aP, cen1)
                                nc.vector.tensor_sub(dM[:, si], aM, cen1)
                                nc.vector.tensor_mul(dP[:, si], dP[:, si], dM[:, si])
                                nc.vector.tensor_tensor(
                                    M[:, si], M[:, si], dP[:, si],
                                    op=mybir.AluOpType.min,
                                )
                            continue
                        nc.vector.tensor_sub(dP, rd2(dy, dx), cen2)
                        nc.vector.tensor_sub(dM, rd2(-dy, -dx), cen2)
                        nc.vector.tensor_mul(dP, dP, dM)
                        nc.vector.tensor_tensor(M, M, dP, op=mybir.AluOpType.min)
                    # per-partition sum / sumsq of this chunk (ScalarE).
                    for si, s in enumerate(SCALES):
                        if last_slab and si == 1:
                            # Copy to scratch, not in place: an in-place Copy
                            # WRITES M, which would serialize the DVE stt
                            # below (a reader of M) behind the Act queue.
                            # The accumulated sum is identical either way.
                            cps = sqp.tile(
                                [128, CR, W], bf, tag="cps", name="cps", bufs=1
                            )
                            nc.scalar.activation(
                                cps, M[:, si], mybir.ActivationFunctionType.Copy,
                                accum_out=acc[:, 2 * si, step : step + 1],
                            )
                        else:
                            nc.scalar.activation(
                                M[:, si], M[:, si],
                                mybir.ActivationFunctionType.Copy,
                                accum_out=acc[:, 2 * si, step : step + 1],
                            )
                        nc.sync.dma_start(
                            out=m_out[s][blk, :, step * CR : step * CR + CR, :],
                            in_=M[:, si],
                        )
                        if last_slab and si == 1:
                            # Kernel tail: square-sum on the DVE (stt with
                            # accum) with private scratch + accum tiles.
                            sqv = sqp.tile(
                                [128, CR, W], bf, tag="sqv", name="sqv", bufs=1
                            )
                            acc2 = sqp.tile(
                                [128, 1], f32, tag="acc2", name="acc2", bufs=1
                            )
                            nc.vector.scalar_tensor_tensor(
                                out=sqv, in0=M[:, si], scalar=1.0, in1=M[:, si],
                                op0=mybir.AluOpType.mult,
                                op1=mybir.AluOpType.mult,
                                accum_out=acc2,
                            )
                            nc.sync.dma_start(
                                out=stats[blk][:, 2 * si + 1, step : step + 1],
                                in_=acc2,
                            )
                        else:
                            sq = sqp.tile(
                                [128, CR, W], bf, tag="sq", name="sq", bufs=1
                            )
                            # Square into a scratch tile (not in place) so it
                            # doesn't have to wait for the M DMA-out to finish.
                            nc.scalar.activation(
                                sq, M[:, si], mybir.ActivationFunctionType.Square,
                                accum_out=acc[:, 2 * si + 1, step : step + 1],
                            )
                if blk == NBLK - 1:
                    # The [3, NSTEPS-1] slot was written directly from the
                    # DVE stt's accumulator; exclude it here so this later
                    # queue entry doesn't clobber it with the stale tile.
                    nc.sync.dma_start(out=stats[blk][:, 0:3, :], in_=acc[:, 0:3, :])
                    nc.sync.dma_start(
                        out=stats[blk][:, 3:4, 0 : NSTEPS - 1],
                        in_=acc[:, 3:4, 0 : NSTEPS - 1],
                    )
                else:
                    nc.sync.dma_start(out=stats[blk], in_=acc)
    return nc


def _build_pass_b():
    import concourse.bacc as bacc
    import concourse.tile as tile
    from concourse import mybir

    nc = bacc.Bacc()
    bf = mybir.dt.bfloat16
    f32 = mybir.dt.float32

    m13 = nc.declare_dram_parameter("m13", [NBLK, 128, ROWS, W], bf, isOutput=False)
    m17 = nc.declare_dram_parameter("m17", [NBLK, 128, ROWS, W], bf, isOutput=False)
    # per plane: [A13, A17, D, pad]
    coef = nc.declare_dram_parameter("coef", [NBLK, 128, 4], f32, isOutput=False)
    out = nc.declare_dram_parameter("out", [NBLK, 128, ROWS, W], bf, isOutput=True)

    # Uneven chunks: a small first chunk shortens the ramp (less data before
    # the first compute), a small last chunk shortens the exposed write tail.
    CHUNKS = (12, 24, 12)
    assert sum(CHUNKS) == ROWS
    with tile.TileContext(nc) as tc:
        with (
            tc.tile_pool(name="cf", bufs=1) as cfp,
            tc.tile_pool(name="io", bufs=2) as io,
        ):
            cf = {}
            for blk in range(NBLK):
                cf[blk] = cfp.tile([128, 4], f32, tag=f"cf{blk}", name=f"cf{blk}")
                nc.sync.dma_start(out=cf[blk], in_=coef[blk])
            ci = 0
            for blk in range(NBLK):
                r0 = 0
                for cr_b in CHUNKS:
                    t13 = io.tile([128, cr_b, W], bf, tag="t13", name="t13")
                    t17 = io.tile([128, cr_b, W], bf, tag="t17", name="t17")
                    # ALL reads ride the sync queue and ALL writes the scalar
                    # queue.  DMA queues execute in order, so a write (which
                    # waits on compute) queued ahead of the next chunk's read
                    # would stall that read — keeping directions on separate
                    # queues means reads stream back-to-back at full rate.
                    nc.sync.dma_start(out=t13, in_=m13[blk, :, r0 : r0 + cr_b, :])
                    nc.sync.dma_start(out=t17, in_=m17[blk, :, r0 : r0 + cr_b, :])
                    # u = A13*m13 + D  (DVE tensor_scalar, 4x mode)
                    u = io.tile([128, cr_b, W], bf, tag="u", name="u")
                    nc.vector.tensor_scalar(
                        out=u, in0=t13,
                        scalar1=cf[blk][:, 0:1], scalar2=cf[blk][:, 2:3],
                        op0=mybir.AluOpType.mult, op1=mybir.AluOpType.add,
                    )
                    # w = A17*m17 (4x); o = w + u (tensor_tensor, 2x).
                    # scalar_tensor_tensor would fuse these but its firmware
                    # runs at 1x, so two stock ops are faster.
                    w = io.tile([128, cr_b, W], bf, tag="w", name="w")
                    nc.vector.tensor_scalar(
                        out=w, in0=t17, scalar1=cf[blk][:, 1:2], scalar2=None,
                        op0=mybir.AluOpType.mult,
                    )
                    o = io.tile([128, cr_b, W], bf, tag="o", name="o")
                    nc.vector.tensor_tensor(o, w, u, op=mybir.AluOpType.add)
                    nc.scalar.dma_start(out=out[blk, :, r0 : r0 + cr_b, :], in_=o)
                    r0 += cr_b
                    ci += 1
    return nc


def _shards_from_cen(cen):
    """Build per-core bf16 halo'd shards cenE: [NBLK,128,SR,SW]."""
    pl = np.ascontiguousarray(cen.reshape(P, H, W)).astype(BF16)
    colsE = (np.arange(-CHALO, W + CHALO)) % W
    shards = []
    for k in range(NCORES):
        rows = (np.arange(-RHALO, ROWS + RHALO) + k * ROWS) % H
        sub = pl[:, rows, :]                       # [P, SR, W]
        e = sub[:, :, colsE].reshape(NBLK, 128, SR, SW)
        shards.append(np.ascontiguousarray(e))
    return shards


def _host_glue(stats_list, bn1_g, bn1_b, bn2_g, bn2_b,
               td_w1, td_b1, td_g1, td_be1, td_w2, td_b2, td_g2, td_be2,
               bu_w1, bu_b1, bu_g1, bu_be1, bu_w2, bu_b2, bu_g2, bu_be2):
    """Combine per-core stats, run BN + SE exactly, return per-plane coefs."""
    f8 = np.float64
    # stats_list[k]: [NBLK, 128, 4, NSTEPS] -> global [P, 4]
    tot = np.zeros((P, 4), f8)
    for st in stats_list:
        tot += st.astype(f8).sum(axis=3).reshape(P, 4)
    sum13 = tot[:, 0].reshape(B, C)
    sq13 = tot[:, 1].reshape(B, C)
    sum17 = tot[:, 2].reshape(B, C)
    sq17 = tot[:, 3].reshape(B, C)

    n = B * H * W

    def bn_affine(sm, sq, g, b):
        mean = sm.sum(0) / n
        var = sq.sum(0) / n - mean * mean
        a = g.astype(f8) / np.sqrt(var + EPS)
        return a, b.astype(f8) - mean * a

    a1, b1 = bn_affine(sum13, sq13, bn1_g, bn1_b)   # BN for pcm13
    a2, b2 = bn_affine(sum17, sq17, bn2_g, bn2_b)   # BN for pcm17

    # (H,W)-pooled normalized pcm per (b,c)
    p13 = a1[None, :] * (sum13 / (H * W)) + b1[None, :]
    p17 = a2[None, :] * (sum17 / (H * W)) + b2[None, :]

    def se(p, w1, bb1, g1, be1, w2, bb2, g2, be2):
        y = p @ w1.astype(f8).T + bb1.astype(f8)[None, :]
        mu, v = y.mean(0), y.var(0)
        y = (y - mu) / np.sqrt(v + EPS) * g1.astype(f8) + be1.astype(f8)
        y = np.maximum(y, 0.0)
        z = y @ w2.astype(f8).T + bb2.astype(f8)[None, :]
        mu, v = z.mean(0), z.var(0)
        z = (z - mu) / np.sqrt(v + EPS) * g2.astype(f8) + be2.astype(f8)
        return 1.0 / (1.0 + np.exp(-z))

    td_wei = se(p17, td_w1, td_b1, td_g1, td_be1, td_w2, td_b2, td_g2, td_be2)
    bu_wei = se(p13, bu_w1, bu_b1, bu_g1, bu_be1, bu_w2, bu_b2, bu_g2, bu_be2)

    A13 = td_wei * a1[None, :]
    A17 = bu_wei * a2[None, :]
    D = td_wei * b1[None, :] + bu_wei * b2[None, :]
    coef = np.zeros((P, 4), np.float32)
    coef[:, 0] = A13.reshape(P)
    coef[:, 1] = A17.reshape(P)
    coef[:, 2] = D.reshape(P)
    return coef.reshape(NBLK, 128, 4)


def _run(nc, in_maps, trace=False):
    from concourse.bass_utils import run_bass_kernel_spmd

    return run_bass_kernel_spmd(nc, in_maps, list(range(NCORES)), trace=trace)


def kernel(cen, bn1_g, bn1_b, bn2_g, bn2_b,
           td_w1, td_b1, td_g1, td_be1, td_w2, td_b2, td_g2, td_be2,
           bu_w1, bu_b1, bu_g1, bu_be1, bu_w2, bu_b2, bu_g2, bu_be2):
    cen = np.asarray(cen, np.float32)

    if "pass_a" not in _cache:
        nca = _build_pass_a()
        nca.compile()
        _cache["pass_a"] = nca
    if "pass_b" not in _cache:
        ncb = _build_pass_b()
        ncb.compile()
        _cache["pass_b"] = ncb

    shards = _shards_from_cen(cen)
    in_a = [{"cenE": e} for e in shards]
    res_a = _run(_cache["pass_a"], in_a).results

    coef = _host_glue(
        [r["stats"] for r in res_a],
        bn1_g, bn1_b, bn2_g, bn2_b,
        td_w1, td_b1, td_g1, td_be1, td_w2, td_b2, td_g2, td_be2,
        bu_w1, bu_b1, bu_g1, bu_be1, bu_w2, bu_b2, bu_g2, bu_be2,
    )

    in_b = [
        {"m13": r["m13"], "m17": r["m17"], "coef": coef} for r in res_a
    ]
    res_b = _run(_cache["pass_b"], in_b).results

    out = np.empty((P, H, W), np.float32)
    for k in range(NCORES):
        out[:, k * ROWS : (k + 1) * ROWS, :] = (
            res_b[k]["out"].reshape(P, ROWS, W).astype(np.float32)
        )
    return out.reshape(B, C, H, W)
